# revision 23
# baseline (speedup 1.0000x reference)
"""Trainium2 Bass kernel for nn_Attention_25168508354564 (sparse_attention).

Reference computation (B=2, T=2048, H=16, dh=64, D=1024):
    qkv = query @ Wqkv;  q,k,v split, per-head
    S   = (q @ k^T) * dh^-0.5, causal+pad masked (-inf)          -> attn_score
    P   = softmax(log_softmax(S) + log(prior + tiny))            -> attn_prob
        = softmax(S + log(prior + tiny))   (row-constant shift cancels)
    out = (P @ v) @ Wo                                           -> out

Sharding: 8 cores = (batch b = core//4) x (head group g = core%4, 4 heads).
Each core computes its 4 heads' score/prob rows plus a partial out^T;
host reshapes shards (zero-copy) and sums the 4 partial outputs per batch.

Self-contained: only needs /opt/trn_rl_repo (staged in the container image).
"""

import sys

sys.path.insert(0, "/opt/trn_rl_repo")

import numpy as np

import concourse.bass as bass
import concourse.mybir as mybir
import concourse.tile as tile
from concourse import bass2jax
from concourse.masks import make_causal_mask, make_identity, make_lower_triangular

F32 = mybir.dt.float32
P = 128
T = 2048
D = 1024
DH = 64
NH = 16
HPC = 4  # heads per core
NB = T // P  # 16 row blocks
SCALE = DH ** -0.5
TINY = float(np.finfo(np.float32).tiny)
NEG_BIG = -1e30  # causal mask for the exp path (must stay multipliable: no -inf)

F32R = mybir.dt.float32r
F32R_QKV = True   # qkv projection matmuls in fp32r (4x faster PE)
F32R_OUT = True   # output projection matmuls in fp32r
F32R_S = True    # score matmuls in fp32r (direct output; off for precision)

MAX_WAITS = 1


def split_sync_waits(nc):
    """walrus in this container rejects instructions carrying more than one
    sync-wait; hoist extra waits onto preceding same-engine NOPs."""
    for f in nc.m.functions:
        for bb in f.blocks:
            new_insts = []
            for inst in bb.instructions:
                si = inst.sync_info
                waits = list(si.on_wait) if si and si.on_wait else []
                if len(waits) > MAX_WAITS:
                    head, keep = waits[:-MAX_WAITS], waits[-MAX_WAITS:]
                    for i in range(0, len(head), MAX_WAITS):
                        nop = mybir.InstNoOp(
                            name=f"{inst.name}-wsplit{i}",
                            engine=inst.engine,
                            sync_info=mybir.SyncInfo(
                                on_wait=head[i : i + MAX_WAITS], on_update=[]
                            ),
                            bass_nofuse=True,
                        )
                        new_insts.append(nop)
                    si.on_wait = keep
                new_insts.append(inst)
            bb.instructions[:] = new_insts


def build_nc(T=T, split=True, reps=1):
    NB = T // P
    nc = bass.Bass(target_bir_lowering=False)

    xT = nc.declare_dram_parameter("xT", [D, T], F32, isOutput=False)
    wqkv = nc.declare_dram_parameter("wqkv", [D, 3 * HPC * DH], F32, isOutput=False)
    wo = nc.declare_dram_parameter("wo", [HPC * DH, D], F32, isOutput=False)
    prior = nc.declare_dram_parameter("prior", [T, T], F32, isOutput=False)
    idrep = nc.declare_dram_parameter("idrep", [P, DH], F32, isOutput=False)
    score = nc.declare_dram_parameter("score", [HPC, T, T], F32, isOutput=True)
    prob = nc.declare_dram_parameter("prob", [HPC, T, T], F32, isOutput=True)
    outT = nc.declare_dram_parameter("outT", [D, T], F32, isOutput=True)

    Exp = mybir.ActivationFunctionType.Exp
    Ln = mybir.ActivationFunctionType.Ln

    with tile.TileContext(nc) as tc:
      for _rep in range(reps):
        with tc.tile_pool(name="singles", bufs=1) as singles:
            identity = singles.tile([P, P], F32)
            make_identity(nc, identity)
            # 0 on/below diagonal, mask above: one for the exp path (finite so
            # the identity-matmul accumulate can't produce 0*inf), one exact
            # -inf for the score output.
            tri_inf = singles.tile([P, P], F32)
            make_causal_mask(nc, tri_inf, mask_val=float("-inf"))
            qk_dt = F32R if F32R_S else F32
            # memset/affine_select can't target f32r tiles; build in f32 and
            # cast-copy (rounds to f32r, exact for 0/1/-1e30)
            scratch = singles.tile([P, P], F32)
            make_causal_mask(nc, scratch, mask_val=NEG_BIG)
            tri_exp = singles.tile([P, P], qk_dt)
            nc.vector.tensor_copy(out=tri_exp, in_=scratch)
            identity_r = singles.tile([P, P], qk_dt)
            nc.vector.tensor_copy(out=identity_r, in_=identity)
            # identity replicated per 64-row half: valid transpose rhs at
            # base_partition 0 and 64
            neginf_t = singles.tile([P, T - P], F32)
            nc.vector.memset(neginf_t, float("-inf"))
            zero_t = singles.tile([P, T - P], F32)
            nc.vector.memset(zero_t, 0.0)
            tiny_bias = singles.tile([P, 1], F32)
            nc.vector.memset(tiny_bias, TINY)
            id64 = singles.tile([P, DH], F32)
            nc.sync.dma_start(out=id64, in_=idrep[:, :])

            qkT = singles.tile([P, 4, T], qk_dt)  # [q0 q1 k0 k1] m-tiles
            vT = singles.tile([P, 2, T], F32)  # [v0 v1] m-tiles
            v_ext = singles.tile([P, NB, HPC, DH], F32)  # v[k,d] per head
            # y^T [256, T]; fp32r so the output projection runs 1 cyc/row
            yT_all = singles.tile([P, 2, T], F32R if F32R_OUT else F32)

            # Phase A: qkv^T = wqkv^T @ x^T  ([768, T], m-tiled by 128)
            with (
                tc.tile_pool(name="phA", bufs=1) as phA,
                tc.tile_pool(name="psA", bufs=4, space="PSUM") as psA,
            ):
                in_dt = F32R if F32R_QKV else F32
                in_dma = nc.gpsimd.dma_start if F32R_QKV else nc.sync.dma_start
                xt = phA.tile([P, 8, T], in_dt)
                in_dma(out=xt, in_=xT.rearrange("(ko p) n -> p ko n", p=P))
                wq = phA.tile([P, 8, 3 * HPC * DH], in_dt)
                in_dma(out=wq, in_=wqkv.rearrange("(ko p) m -> p ko m", p=P))
                for m in range(6):
                    for n in range(T // 512):
                        ps = psA.tile([P, 512], F32)
                        for k in range(8):
                            nc.tensor.matmul(
                                ps,
                                lhsT=wq[:, k, m * P : (m + 1) * P],
                                rhs=xt[:, k, n * 512 : (n + 1) * 512],
                                start=(k == 0),
                                stop=(k == 7),
                            )
                        dst = (
                            qkT[:, m, n * 512 : (n + 1) * 512]
                            if m < 4
                            else vT[:, m - 4, n * 512 : (n + 1) * 512]
                        )
                        nc.vector.tensor_copy(out=dst, in_=ps)

            # Phase B: v_ext[k-block j, head h] = v[k, d] via PE transpose of v^T
            with tc.tile_pool(name="psB", bufs=4, space="PSUM") as psB:
                for j in range(NB):
                    for h in range(HPC):
                        base = (h % 2) * DH
                        src = vT[base : base + DH, h // 2, j * P : (j + 1) * P]
                        ps = psB.tile([P, DH], F32)
                        nc.tensor.transpose(ps, src, id64[base : base + DH, :])
                        nc.vector.tensor_copy(out=v_ext[:, j, h, :], in_=ps)

            # Phase C: per (row-block qi, head h)
            with (
                tc.tile_pool(name="prior_p", bufs=2) as prior_p,
                tc.tile_pool(name="logp_p", bufs=2) as logp_p,
                tc.tile_pool(name="score_p", bufs=3) as score_p,
                tc.tile_pool(name="pu_p", bufs=3) as pu_p,
                tc.tile_pool(name="pt_p", bufs=3) as pt_p,
                tc.tile_pool(name="small_p", bufs=4) as small_p,
                tc.tile_pool(name="psS", bufs=4, space="PSUM") as psS,
                tc.tile_pool(name="psT", bufs=2, space="PSUM") as psT,
                tc.tile_pool(name="psY", bufs=2, space="PSUM") as psY,
            ):
                for qi in range(NB):
                    cact = P * (qi + 1)
                    pr = prior_p.tile([P, T], F32)
                    nc.sync.dma_start(
                        out=pr[:, :cact], in_=prior[qi * P : (qi + 1) * P, 0:cact]
                    )
                    # lp = ln(prior + tiny) (+ causal -1e30 inside the diag
                    # block); added into the score PSUM via identity matmul so
                    # exp(S + lp) = prior-weighted unnormalized probability
                    lp = logp_p.tile([P, T], qk_dt)
                    nc.scalar.activation(
                        out=lp[:, :cact], in_=pr[:, :cact], func=Ln, bias=tiny_bias
                    )
                    nc.vector.tensor_add(
                        out=lp[:, qi * P : (qi + 1) * P],
                        in0=lp[:, qi * P : (qi + 1) * P],
                        in1=tri_exp,
                    )
                    for h in range(HPC):
                        base = (h % 2) * DH
                        qm = h // 2
                        km = 2 + h // 2
                        lhs_q = qkT[base : base + DH, qm, qi * P : (qi + 1) * P]
                        sc_row = score_p.tile([P, T], F32)
                        pu_row = pu_p.tile([P, T], F32)
                        sums = small_p.tile([P, 4], F32, tag="sums")
                        rsum = small_p.tile([P, 1], F32, tag="rsum")
                        if cact < T:
                            # dependency-free: scheduler hoists these into DMA
                            # idle (e.g. during the qkv projection)
                            nc.sync.dma_start(
                                out=score[h, qi * P : (qi + 1) * P, cact:T],
                                in_=neginf_t[:, : T - cact],
                            )
                            nc.sync.dma_start(
                                out=prob[h, qi * P : (qi + 1) * P, cact:T],
                                in_=zero_t[:, : T - cact],
                            )
                        CH = 512
                        nchunk = (cact + CH - 1) // CH
                        for ic in range(nchunk):
                            w = min(CH, cact - ic * CH)
                            c0 = ic * CH
                            ps = psS.tile([P, CH], F32)
                            nc.tensor.matmul(
                                ps[:, :w],
                                lhsT=lhs_q,
                                rhs=qkT[base : base + DH, km, c0 : c0 + w],
                                start=True,
                                stop=True,
                            )
                            # raw scores -> sbuf (exact -inf causal mask on the
                            # diagonal block, which sits at the end of the row)
                            wd = w - P if ic == nchunk - 1 else w
                            if wd > 0:
                                nc.scalar.copy(
                                    out=sc_row[:, c0 : c0 + wd], in_=ps[:, :wd]
                                )
                            if ic == nchunk - 1:
                                nc.vector.tensor_add(
                                    out=sc_row[:, qi * P : (qi + 1) * P],
                                    in0=ps[:, w - P : w],
                                    in1=tri_inf,
                                )
                            nc.tensor.matmul(
                                ps[:, :w],
                                lhsT=identity_r,
                                rhs=lp[:, c0 : c0 + w],
                                start=False,
                                stop=True,
                                skip_group_check=True,
                            )
                            nc.scalar.activation(
                                out=pu_row[:, c0 : c0 + w],
                                in_=ps[:, :w],
                                func=Exp,
                                accum_out=sums[:, ic : ic + 1],
                            )
                        rinv = small_p.tile([P, 1], F32, tag="rinv")
                        if nchunk > 1:
                            nc.vector.tensor_reduce(
                                out=rsum,
                                in_=sums[:, :nchunk],
                                axis=mybir.AxisListType.X,
                                op=mybir.AluOpType.add,
                            )
                            nc.vector.reciprocal(rinv, rsum)
                        else:
                            nc.vector.reciprocal(rinv, sums[:, 0:1])

                        # y = (Pu @ v) * rinv  via PE transpose of Pu blocks
                        psy = psY.tile([P, DH], F32)
                        for j4 in range(0, qi + 1, 4):
                            jn = min(4, qi + 1 - j4)
                            pst = psT.tile([P, 512], F32, tag="pst")
                            for jj in range(jn):
                                j = j4 + jj
                                nc.tensor.transpose(
                                    pst[:, jj * P : (jj + 1) * P],
                                    pu_row[:, j * P : (j + 1) * P],
                                    identity,
                                )
                            ptb = pt_p.tile([P, 512], F32)
                            nc.vector.tensor_copy(
                                out=ptb[:, : jn * P], in_=pst[:, : jn * P]
                            )
                            for jj in range(jn):
                                j = j4 + jj
                                nc.tensor.matmul(
                                    psy,
                                    lhsT=ptb[:, jj * P : (jj + 1) * P],
                                    rhs=v_ext[:, j, h, :],
                                    start=(j == 0),
                                    stop=(j == qi),
                                )
                        y_sb = small_p.tile([P, DH], F32, tag="y")
                        nc.vector.tensor_scalar_mul(out=y_sb, in0=psy, scalar1=rinv)
                        # y^T into yT_all
                        pst2 = psT.tile([P, 512], F32, tag="pst")
                        nc.tensor.transpose(pst2[:DH, :P], y_sb, identity)
                        nc.vector.tensor_copy(
                            out=yT_all[base : base + DH, qm, qi * P : (qi + 1) * P],
                            in_=pst2[:DH, :P],
                        )
                        # normalize P in place and ship full rows (tails
                        # pre-set by gpsimd memsets above)
                        nc.vector.tensor_scalar_mul(
                            out=pu_row[:, :cact], in0=pu_row[:, :cact], scalar1=rinv
                        )
                        nc.sync.dma_start(
                            out=prob[h, qi * P : (qi + 1) * P, 0:cact],
                            in_=pu_row[:, :cact],
                        )
                        nc.sync.dma_start(
                            out=score[h, qi * P : (qi + 1) * P, 0:cact],
                            in_=sc_row[:, :cact],
                        )

            # Phase D: outT = wo^T @ y^T  (partial out for this head group)
            with (
                tc.tile_pool(name="phD", bufs=1) as phD,
                tc.tile_pool(name="psD", bufs=4, space="PSUM") as psD,
                tc.tile_pool(name="od", bufs=2) as od,
            ):
                wo_sb = phD.tile([P, 2, D], F32R if F32R_OUT else F32)
                wo_dma = nc.gpsimd.dma_start if F32R_OUT else nc.sync.dma_start
                wo_dma(out=wo_sb, in_=wo.rearrange("(ko p) m -> p ko m", p=P))
                for m in range(8):
                    ot = od.tile([P, T], F32)
                    for n in range(T // 512):
                        ps = psD.tile([P, 512], F32)
                        for k in range(2):
                            nc.tensor.matmul(
                                ps,
                                lhsT=wo_sb[:, k, m * P : (m + 1) * P],
                                rhs=yT_all[:, k, n * 512 : (n + 1) * 512],
                                start=(k == 0),
                                stop=(k == 1),
                            )
                        nc.vector.tensor_copy(
                            out=ot[:, n * 512 : (n + 1) * 512], in_=ps
                        )
                    nc.sync.dma_start(out=outT[m * P : (m + 1) * P, :], in_=ot)

    if split:
        split_sync_waits(nc)
    return nc


class _Runner:
    """Compile once per process; execute via PJRT shard_map on 8 cores."""

    def __init__(self, nc, n_cores=8):
        import jax
        from jax.sharding import Mesh, NamedSharding, PartitionSpec
        from jax.experimental.shard_map import shard_map

        bass2jax.install_neuronx_cc_hook()
        self.jax = jax
        self.nc = nc
        self.n_cores = n_cores
        partition_name = (
            nc.partition_id_tensor.name if nc.partition_id_tensor else None
        )
        in_names, out_names, out_avals, zero_shapes = [], [], [], []
        for alloc in nc.m.functions[0].allocations:
            if not isinstance(alloc, mybir.MemoryLocationSet):
                continue
            name = alloc.memorylocations[0].name
            if alloc.kind == "ExternalInput":
                if name != partition_name:
                    in_names.append(name)
            elif alloc.kind == "ExternalOutput":
                out_names.append(name)
                shape = tuple(alloc.tensor_shape)
                dtype = mybir.dt.np(alloc.dtype)
                out_avals.append(jax.core.ShapedArray(shape, dtype))
                zero_shapes.append((shape, dtype))
        self.in_names = in_names
        self.out_names = out_names
        self.out_avals = out_avals
        self.zero_shapes = zero_shapes

        bind_in_names = list(in_names) + list(out_names)
        if partition_name is not None:
            bind_in_names.append(partition_name)

        def _body(*args):
            operands = list(args)
            if partition_name is not None:
                operands.append(bass2jax.partition_id_tensor())
            outs = bass2jax._bass_exec_p.bind(
                *operands,
                out_avals=tuple(out_avals),
                in_names=tuple(bind_in_names),
                out_names=tuple(out_names),
                lowering_input_output_aliases=(),
                sim_require_finite=True,
                sim_require_nnan=True,
                nc=nc,
            )
            return tuple(outs)

        devices = jax.devices()[:n_cores]
        assert len(devices) == n_cores, f"need {n_cores} devices"
        self.mesh = Mesh(np.asarray(devices), ("core",))
        nin, nout = len(in_names), len(out_names)
        self.fn = jax.jit(
            shard_map(
                _body,
                mesh=self.mesh,
                in_specs=(PartitionSpec("core"),) * (nin + nout),
                out_specs=(PartitionSpec("core"),) * nout,
                check_rep=False,
            )
        )
        self.sharding = NamedSharding(self.mesh, PartitionSpec("core"))

    def put_inputs(self, in_maps):
        concat = [
            np.concatenate([np.asarray(m[name]) for m in in_maps], axis=0)
            for name in self.in_names
        ]
        zeros = [
            np.zeros((self.n_cores * s[0], *s[1:]), d) for s, d in self.zero_shapes
        ]
        return [self.jax.device_put(a, self.sharding) for a in concat + zeros]

    def run(self, args):
        outs = self.fn(*args)
        self.jax.block_until_ready(outs)
        return {name: outs[i] for i, name in enumerate(self.out_names)}


_RUNNER = None


def _get_runner():
    global _RUNNER
    if _RUNNER is None:
        _RUNNER = _Runner(build_nc(), 8)
    return _RUNNER


def make_in_maps(query, attn_prior, Wqkv, Wo):
    """Build the 8 per-core input dicts from full inputs."""
    query = np.asarray(query, dtype=np.float32)
    attn_prior = np.asarray(attn_prior, dtype=np.float32)
    Wqkv = np.asarray(Wqkv, dtype=np.float32)
    Wo = np.asarray(Wo, dtype=np.float32)
    xTs = [np.ascontiguousarray(query[b].T) for b in range(2)]
    in_maps = []
    for c in range(8):
        b, g = divmod(c, 4)
        cols = slice(g * HPC * DH, (g + 1) * HPC * DH)  # 256 cols of this group
        wq = Wqkv[:, 0 * NH * DH :][:, cols] * np.float32(SCALE)
        wk = Wqkv[:, 1 * NH * DH :][:, cols]
        wv = Wqkv[:, 2 * NH * DH :][:, cols]
        in_maps.append(
            {
                "idrep": np.tile(np.eye(DH, dtype=np.float32), (2, 1)),
                "xT": xTs[b],
                "wqkv": np.ascontiguousarray(
                    np.concatenate([wq, wk, wv], axis=1)
                ),
                "wo": np.ascontiguousarray(Wo[cols, :]),
                "prior": attn_prior[b],
            }
        )
    return in_maps


def _numpy_fallback(query, query_mask, attn_prior, Wqkv, Wo):
    """Masked (general) path — mirrors the reference in numpy. Only used when
    query_mask is not all-ones, which the problem spec never produces."""
    q_ = query.astype(np.float32)
    Bq, Tq, _ = q_.shape
    qkv = q_ @ Wqkv
    q, k, v = np.split(qkv, 3, axis=-1)
    q = q.reshape(Bq, Tq, NH, DH).transpose(0, 2, 1, 3)
    k = k.reshape(Bq, Tq, NH, DH).transpose(0, 2, 1, 3)
    v = v.reshape(Bq, Tq, NH, DH).transpose(0, 2, 1, 3)
    s = np.einsum("bhqd,bhkd->bhqk", q, k) * SCALE
    pad = query_mask[:, None, :, None] & query_mask[:, None, None, :]
    s = np.where(pad, s, -np.inf)
    causal = np.tril(np.ones((Tq, Tq), dtype=bool))
    s = np.where(causal[None, None], s, -np.inf)
    smax = np.max(s, axis=-1, keepdims=True)
    smax = np.where(np.isfinite(smax), smax, 0.0)
    e = np.exp(s - smax)
    lse = np.log(np.sum(e, axis=-1, keepdims=True)) + smax
    slog = s - lse
    t = slog + np.log(attn_prior[:, :Tq][:, None] + TINY)
    tmax = np.max(t, axis=-1, keepdims=True)
    tmax = np.where(np.isfinite(tmax), tmax, 0.0)
    et = np.exp(t - tmax)
    p = et / np.sum(et, axis=-1, keepdims=True)
    p = np.where(pad, p, 0.0)
    y = np.einsum("bhqk,bhkd->bhqd", p, v)
    y = y.transpose(0, 2, 1, 3).reshape(Bq, Tq, NH * DH)
    out = y @ Wo
    return (
        out.astype(np.float32),
        p.astype(np.float32),
        s.astype(np.float32),
    )


def kernel(query, query_mask, attn_prior, Wqkv, Wo):
    query = np.asarray(query)
    query_mask = np.asarray(query_mask)
    attn_prior = np.asarray(attn_prior)
    Wqkv = np.asarray(Wqkv)
    Wo = np.asarray(Wo)
    if not query_mask.all():
        return _numpy_fallback(query, query_mask, attn_prior, Wqkv, Wo)

    r = _get_runner()
    in_maps = make_in_maps(query, attn_prior, Wqkv, Wo)
    args = r.put_inputs(in_maps)
    outs = r.run(args)

    # (8*4, T, T): cores ordered (b, g) with heads ascending -> zero-copy views
    score = np.asarray(outs["score"]).reshape(2, NH, T, T)
    prob = np.asarray(outs["prob"]).reshape(2, NH, T, T)
    outT = np.asarray(outs["outT"]).reshape(8, D, T)
    out = np.stack(
        [
            (outT[0] + outT[1] + outT[2] + outT[3]).T,
            (outT[4] + outT[5] + outT[6] + outT[7]).T,
        ]
    ).astype(np.float32)
    return out, prob, score


# revision 24
# speedup vs baseline: 20.5328x; 20.5328x over previous
"""Trainium2 Bass kernel for nn_Attention_25168508354564 (sparse_attention).

Reference computation (B=2, T=2048, H=16, dh=64, D=1024):
    qkv = query @ Wqkv;  q,k,v split, per-head
    S   = (q @ k^T) * dh^-0.5, causal+pad masked (-inf)          -> attn_score
    P   = softmax(log_softmax(S) + log(prior + tiny))            -> attn_prob
        = softmax(S + log(prior + tiny))   (row-constant shift cancels)
    out = (P @ v) @ Wo                                           -> out

Sharding: 8 cores = (batch b = core//4) x (head group g = core%4, 4 heads).
Each core computes its 4 heads' score/prob rows plus a partial out^T;
host reshapes shards (zero-copy) and sums the 4 partial outputs per batch.

Self-contained: only needs /opt/trn_rl_repo (staged in the container image).
"""

import sys

sys.path.insert(0, "/opt/trn_rl_repo")

import numpy as np

import concourse.bass as bass
import concourse.mybir as mybir
import concourse.tile as tile
from concourse import bass2jax
from concourse.masks import make_causal_mask, make_identity

F32 = mybir.dt.float32
P = 128
T = 2048
D = 1024
DH = 64
NH = 16
HPC = 4  # heads per core
NB = T // P  # 16 row blocks
SCALE = DH ** -0.5
TINY = float(np.finfo(np.float32).tiny)
NEG_BIG = -1e30  # causal mask for the exp path (must stay multipliable: no -inf)

F32R = mybir.dt.float32r
F32R_QKV = True   # qkv projection matmuls in fp32r (4x faster PE)
F32R_OUT = True   # output projection matmuls in fp32r
F32R_S = True    # score matmuls in fp32r (direct output; off for precision)

MAX_WAITS = 1


def split_sync_waits(nc):
    """walrus in this container rejects instructions carrying more than one
    sync-wait; hoist extra waits onto preceding same-engine NOPs."""
    for f in nc.m.functions:
        for bb in f.blocks:
            new_insts = []
            for inst in bb.instructions:
                si = inst.sync_info
                waits = list(si.on_wait) if si and si.on_wait else []
                if len(waits) > MAX_WAITS:
                    head, keep = waits[:-MAX_WAITS], waits[-MAX_WAITS:]
                    for i in range(0, len(head), MAX_WAITS):
                        nop = mybir.InstNoOp(
                            name=f"{inst.name}-wsplit{i}",
                            engine=inst.engine,
                            sync_info=mybir.SyncInfo(
                                on_wait=head[i : i + MAX_WAITS], on_update=[]
                            ),
                            bass_nofuse=True,
                        )
                        new_insts.append(nop)
                    si.on_wait = keep
                new_insts.append(inst)
            bb.instructions[:] = new_insts


def build_nc(T=T, split=True, reps=1):
    NB = T // P
    nc = bass.Bass(target_bir_lowering=False)

    xT = nc.declare_dram_parameter("xT", [D, T], F32, isOutput=False)
    wqkv = nc.declare_dram_parameter("wqkv", [D, 3 * HPC * DH], F32, isOutput=False)
    wo = nc.declare_dram_parameter("wo", [HPC * DH, D], F32, isOutput=False)
    prior = nc.declare_dram_parameter("prior", [T, T], F32, isOutput=False)
    idrep = nc.declare_dram_parameter("idrep", [P, DH], F32, isOutput=False)
    score = nc.declare_dram_parameter("score", [HPC, T, T], F32, isOutput=True)
    prob = nc.declare_dram_parameter("prob", [HPC, T, T], F32, isOutput=True)
    outT = nc.declare_dram_parameter("outT", [D, T], F32, isOutput=True)

    Exp = mybir.ActivationFunctionType.Exp
    Ln = mybir.ActivationFunctionType.Ln

    with tile.TileContext(nc) as tc:
      for _rep in range(reps):
        with tc.tile_pool(name="singles", bufs=1) as singles:
            identity = singles.tile([P, P], F32)
            make_identity(nc, identity)
            # 0 on/below diagonal, mask above: one for the exp path (finite so
            # the identity-matmul accumulate can't produce 0*inf), one exact
            # -inf for the score output.
            tri_inf = singles.tile([P, P], F32)
            make_causal_mask(nc, tri_inf, mask_val=float("-inf"))
            qk_dt = F32R if F32R_S else F32
            # memset/affine_select can't target f32r tiles; build in f32 and
            # cast-copy (rounds to f32r, exact for 0/1/-1e30)
            scratch = singles.tile([P, P], F32)
            make_causal_mask(nc, scratch, mask_val=NEG_BIG)
            tri_exp = singles.tile([P, P], qk_dt)
            nc.vector.tensor_copy(out=tri_exp, in_=scratch)
            identity_r = singles.tile([P, P], qk_dt)
            nc.vector.tensor_copy(out=identity_r, in_=identity)
            # identity replicated per 64-row half: valid transpose rhs at
            # base_partition 0 and 64
            neginf_t = singles.tile([P, T - P], F32)
            nc.vector.memset(neginf_t, float("-inf"))
            zero_t = singles.tile([P, T - P], F32)
            nc.vector.memset(zero_t, 0.0)
            tiny_bias = singles.tile([P, 1], F32)
            nc.vector.memset(tiny_bias, TINY)
            id64 = singles.tile([P, DH], F32)
            nc.sync.dma_start(out=id64, in_=idrep[:, :])

            qkT = singles.tile([P, 4, T], qk_dt)  # [q0 q1 k0 k1] m-tiles
            vT = singles.tile([P, 2, T], F32)  # [v0 v1] m-tiles
            v_ext = singles.tile([P, NB, HPC, DH], F32)  # v[k,d] per head
            # y^T [256, T]; fp32r so the output projection runs 1 cyc/row
            yT_all = singles.tile([P, 2, T], F32R if F32R_OUT else F32)

            # Phase A: qkv^T = wqkv^T @ x^T  ([768, T], m-tiled by 128)
            with (
                tc.tile_pool(name="phA", bufs=1) as phA,
                tc.tile_pool(name="psA", bufs=4, space="PSUM") as psA,
            ):
                in_dt = F32R if F32R_QKV else F32
                in_dma = nc.gpsimd.dma_start if F32R_QKV else nc.sync.dma_start
                xt = phA.tile([P, 8, T], in_dt)
                in_dma(out=xt, in_=xT.rearrange("(ko p) n -> p ko n", p=P))
                wq = phA.tile([P, 8, 3 * HPC * DH], in_dt)
                in_dma(out=wq, in_=wqkv.rearrange("(ko p) m -> p ko m", p=P))
                for m in range(6):
                    for n in range(T // 512):
                        ps = psA.tile([P, 512], F32)
                        for k in range(8):
                            nc.tensor.matmul(
                                ps,
                                lhsT=wq[:, k, m * P : (m + 1) * P],
                                rhs=xt[:, k, n * 512 : (n + 1) * 512],
                                start=(k == 0),
                                stop=(k == 7),
                            )
                        dst = (
                            qkT[:, m, n * 512 : (n + 1) * 512]
                            if m < 4
                            else vT[:, m - 4, n * 512 : (n + 1) * 512]
                        )
                        nc.vector.tensor_copy(out=dst, in_=ps)

            # Phase B: v_ext[k-block j, head h] = v[k, d] via PE transpose of v^T
            with tc.tile_pool(name="psB", bufs=4, space="PSUM") as psB:
                for j in range(NB):
                    for h in range(HPC):
                        base = (h % 2) * DH
                        src = vT[base : base + DH, h // 2, j * P : (j + 1) * P]
                        ps = psB.tile([P, DH], F32)
                        nc.tensor.transpose(ps, src, id64[base : base + DH, :])
                        nc.vector.tensor_copy(out=v_ext[:, j, h, :], in_=ps)

            # Phase C: per (row-block qi, head h)
            with (
                tc.tile_pool(name="prior_p", bufs=2) as prior_p,
                tc.tile_pool(name="logp_p", bufs=2) as logp_p,
                tc.tile_pool(name="score_p", bufs=3) as score_p,
                tc.tile_pool(name="pu_p", bufs=3) as pu_p,
                tc.tile_pool(name="pt_p", bufs=3) as pt_p,
                tc.tile_pool(name="small_p", bufs=4) as small_p,
                tc.tile_pool(name="psS", bufs=4, space="PSUM") as psS,
                tc.tile_pool(name="psT", bufs=2, space="PSUM") as psT,
                tc.tile_pool(name="psY", bufs=2, space="PSUM") as psY,
            ):
                for qi in range(NB):
                    cact = P * (qi + 1)
                    pr = prior_p.tile([P, T], F32)
                    nc.sync.dma_start(
                        out=pr[:, :cact], in_=prior[qi * P : (qi + 1) * P, 0:cact]
                    )
                    # lp = ln(prior + tiny) (+ causal -1e30 inside the diag
                    # block); added into the score PSUM via identity matmul so
                    # exp(S + lp) = prior-weighted unnormalized probability
                    lp = logp_p.tile([P, T], qk_dt)
                    nc.scalar.activation(
                        out=lp[:, :cact], in_=pr[:, :cact], func=Ln, bias=tiny_bias
                    )
                    nc.vector.tensor_add(
                        out=lp[:, qi * P : (qi + 1) * P],
                        in0=lp[:, qi * P : (qi + 1) * P],
                        in1=tri_exp,
                    )
                    for h in range(HPC):
                        base = (h % 2) * DH
                        qm = h // 2
                        km = 2 + h // 2
                        lhs_q = qkT[base : base + DH, qm, qi * P : (qi + 1) * P]
                        sc_row = score_p.tile([P, T], F32)
                        pu_row = pu_p.tile([P, T], F32)
                        sums = small_p.tile([P, 4], F32, tag="sums")
                        rsum = small_p.tile([P, 1], F32, tag="rsum")
                        if cact < T:
                            # dependency-free: scheduler hoists these into DMA
                            # idle (e.g. during the qkv projection)
                            nc.sync.dma_start(
                                out=score[h, qi * P : (qi + 1) * P, cact:T],
                                in_=neginf_t[:, : T - cact],
                            )
                            nc.sync.dma_start(
                                out=prob[h, qi * P : (qi + 1) * P, cact:T],
                                in_=zero_t[:, : T - cact],
                            )
                        CH = 512
                        nchunk = (cact + CH - 1) // CH
                        for ic in range(nchunk):
                            w = min(CH, cact - ic * CH)
                            c0 = ic * CH
                            ps = psS.tile([P, CH], F32)
                            nc.tensor.matmul(
                                ps[:, :w],
                                lhsT=lhs_q,
                                rhs=qkT[base : base + DH, km, c0 : c0 + w],
                                start=True,
                                stop=True,
                            )
                            # raw scores -> sbuf (exact -inf causal mask on the
                            # diagonal block, which sits at the end of the row)
                            wd = w - P if ic == nchunk - 1 else w
                            if wd > 0:
                                nc.scalar.copy(
                                    out=sc_row[:, c0 : c0 + wd], in_=ps[:, :wd]
                                )
                            if ic == nchunk - 1:
                                nc.vector.tensor_add(
                                    out=sc_row[:, qi * P : (qi + 1) * P],
                                    in0=ps[:, w - P : w],
                                    in1=tri_inf,
                                )
                            nc.tensor.matmul(
                                ps[:, :w],
                                lhsT=identity_r,
                                rhs=lp[:, c0 : c0 + w],
                                start=False,
                                stop=True,
                                skip_group_check=True,
                            )
                            nc.scalar.activation(
                                out=pu_row[:, c0 : c0 + w],
                                in_=ps[:, :w],
                                func=Exp,
                                accum_out=sums[:, ic : ic + 1],
                            )
                        rinv = small_p.tile([P, 1], F32, tag="rinv")
                        if nchunk > 1:
                            nc.vector.tensor_reduce(
                                out=rsum,
                                in_=sums[:, :nchunk],
                                axis=mybir.AxisListType.X,
                                op=mybir.AluOpType.add,
                            )
                            nc.vector.reciprocal(rinv, rsum)
                        else:
                            nc.vector.reciprocal(rinv, sums[:, 0:1])

                        # y = (Pu @ v) * rinv  via PE transpose of Pu blocks
                        psy = psY.tile([P, DH], F32)
                        for j4 in range(0, qi + 1, 4):
                            jn = min(4, qi + 1 - j4)
                            pst = psT.tile([P, 512], F32, tag="pst")
                            for jj in range(jn):
                                j = j4 + jj
                                nc.tensor.transpose(
                                    pst[:, jj * P : (jj + 1) * P],
                                    pu_row[:, j * P : (j + 1) * P],
                                    identity,
                                )
                            ptb = pt_p.tile([P, 512], F32)
                            nc.vector.tensor_copy(
                                out=ptb[:, : jn * P], in_=pst[:, : jn * P]
                            )
                            for jj in range(jn):
                                j = j4 + jj
                                nc.tensor.matmul(
                                    psy,
                                    lhsT=ptb[:, jj * P : (jj + 1) * P],
                                    rhs=v_ext[:, j, h, :],
                                    start=(j == 0),
                                    stop=(j == qi),
                                )
                        y_sb = small_p.tile([P, DH], F32, tag="y")
                        nc.vector.tensor_scalar_mul(out=y_sb, in0=psy, scalar1=rinv)
                        # y^T into yT_all
                        pst2 = psT.tile([P, 512], F32, tag="pst")
                        nc.tensor.transpose(pst2[:DH, :P], y_sb, identity)
                        nc.vector.tensor_copy(
                            out=yT_all[base : base + DH, qm, qi * P : (qi + 1) * P],
                            in_=pst2[:DH, :P],
                        )
                        # normalize P in place and ship full rows (tails
                        # pre-set by gpsimd memsets above)
                        nc.vector.tensor_scalar_mul(
                            out=pu_row[:, :cact], in0=pu_row[:, :cact], scalar1=rinv
                        )
                        nc.sync.dma_start(
                            out=prob[h, qi * P : (qi + 1) * P, 0:cact],
                            in_=pu_row[:, :cact],
                        )
                        nc.sync.dma_start(
                            out=score[h, qi * P : (qi + 1) * P, 0:cact],
                            in_=sc_row[:, :cact],
                        )

            # Phase D: outT = wo^T @ y^T  (partial out for this head group)
            with (
                tc.tile_pool(name="phD", bufs=1) as phD,
                tc.tile_pool(name="psD", bufs=4, space="PSUM") as psD,
                tc.tile_pool(name="od", bufs=2) as od,
            ):
                wo_sb = phD.tile([P, 2, D], F32R if F32R_OUT else F32)
                wo_dma = nc.gpsimd.dma_start if F32R_OUT else nc.sync.dma_start
                wo_dma(out=wo_sb, in_=wo.rearrange("(ko p) m -> p ko m", p=P))
                for m in range(8):
                    ot = od.tile([P, T], F32)
                    for n in range(T // 512):
                        ps = psD.tile([P, 512], F32)
                        for k in range(2):
                            nc.tensor.matmul(
                                ps,
                                lhsT=wo_sb[:, k, m * P : (m + 1) * P],
                                rhs=yT_all[:, k, n * 512 : (n + 1) * 512],
                                start=(k == 0),
                                stop=(k == 1),
                            )
                        nc.vector.tensor_copy(
                            out=ot[:, n * 512 : (n + 1) * 512], in_=ps
                        )
                    nc.sync.dma_start(out=outT[m * P : (m + 1) * P, :], in_=ot)

    if split:
        split_sync_waits(nc)
    return nc


class _Runner:
    """Compile once per process; execute via PJRT shard_map on 8 cores."""

    def __init__(self, nc, n_cores=8):
        import jax
        from jax.sharding import Mesh, NamedSharding, PartitionSpec
        from jax.experimental.shard_map import shard_map

        bass2jax.install_neuronx_cc_hook()
        self.jax = jax
        self.nc = nc
        self.n_cores = n_cores
        partition_name = (
            nc.partition_id_tensor.name if nc.partition_id_tensor else None
        )
        in_names, out_names, out_avals, zero_shapes = [], [], [], []
        for alloc in nc.m.functions[0].allocations:
            if not isinstance(alloc, mybir.MemoryLocationSet):
                continue
            name = alloc.memorylocations[0].name
            if alloc.kind == "ExternalInput":
                if name != partition_name:
                    in_names.append(name)
            elif alloc.kind == "ExternalOutput":
                out_names.append(name)
                shape = tuple(alloc.tensor_shape)
                dtype = mybir.dt.np(alloc.dtype)
                out_avals.append(jax.core.ShapedArray(shape, dtype))
                zero_shapes.append((shape, dtype))
        self.in_names = in_names
        self.out_names = out_names
        self.out_avals = out_avals
        self.zero_shapes = zero_shapes

        bind_in_names = list(in_names) + list(out_names)
        if partition_name is not None:
            bind_in_names.append(partition_name)

        def _body(*args):
            operands = list(args)
            if partition_name is not None:
                operands.append(bass2jax.partition_id_tensor())
            outs = bass2jax._bass_exec_p.bind(
                *operands,
                out_avals=tuple(out_avals),
                in_names=tuple(bind_in_names),
                out_names=tuple(out_names),
                lowering_input_output_aliases=(),
                sim_require_finite=True,
                sim_require_nnan=True,
                nc=nc,
            )
            return tuple(outs)

        devices = jax.devices()[:n_cores]
        assert len(devices) == n_cores, f"need {n_cores} devices"
        self.mesh = Mesh(np.asarray(devices), ("core",))
        nin, nout = len(in_names), len(out_names)
        self.fn = jax.jit(
            shard_map(
                _body,
                mesh=self.mesh,
                in_specs=(PartitionSpec("core"),) * (nin + nout),
                out_specs=(PartitionSpec("core"),) * nout,
                check_rep=False,
            )
        )
        self.sharding = NamedSharding(self.mesh, PartitionSpec("core"))

    def put_inputs(self, in_maps):
        concat = [
            np.concatenate([np.asarray(m[name]) for m in in_maps], axis=0)
            for name in self.in_names
        ]
        zeros = [
            np.zeros((self.n_cores * s[0], *s[1:]), d) for s, d in self.zero_shapes
        ]
        return [self.jax.device_put(a, self.sharding) for a in concat + zeros]

    def run(self, args):
        outs = self.fn(*args)
        self.jax.block_until_ready(outs)
        return {name: outs[i] for i, name in enumerate(self.out_names)}


_RUNNER = None


def _get_runner():
    global _RUNNER
    if _RUNNER is None:
        _RUNNER = _Runner(build_nc(), 8)
    return _RUNNER


def make_in_maps(query, attn_prior, Wqkv, Wo):
    """Build the 8 per-core input dicts from full inputs."""
    query = np.asarray(query, dtype=np.float32)
    attn_prior = np.asarray(attn_prior, dtype=np.float32)
    Wqkv = np.asarray(Wqkv, dtype=np.float32)
    Wo = np.asarray(Wo, dtype=np.float32)
    xTs = [np.ascontiguousarray(query[b].T) for b in range(2)]
    in_maps = []
    for c in range(8):
        b, g = divmod(c, 4)
        cols = slice(g * HPC * DH, (g + 1) * HPC * DH)  # 256 cols of this group
        wq = Wqkv[:, 0 * NH * DH :][:, cols] * np.float32(SCALE)
        wk = Wqkv[:, 1 * NH * DH :][:, cols]
        wv = Wqkv[:, 2 * NH * DH :][:, cols]
        in_maps.append(
            {
                "idrep": np.tile(np.eye(DH, dtype=np.float32), (2, 1)),
                "xT": xTs[b],
                "wqkv": np.ascontiguousarray(
                    np.concatenate([wq, wk, wv], axis=1)
                ),
                "wo": np.ascontiguousarray(Wo[cols, :]),
                "prior": attn_prior[b],
            }
        )
    return in_maps


def _numpy_fallback(query, query_mask, attn_prior, Wqkv, Wo):
    """Masked (general) path — mirrors the reference in numpy. Only used when
    query_mask is not all-ones, which the problem spec never produces."""
    q_ = query.astype(np.float32)
    Bq, Tq, _ = q_.shape
    qkv = q_ @ Wqkv
    q, k, v = np.split(qkv, 3, axis=-1)
    q = q.reshape(Bq, Tq, NH, DH).transpose(0, 2, 1, 3)
    k = k.reshape(Bq, Tq, NH, DH).transpose(0, 2, 1, 3)
    v = v.reshape(Bq, Tq, NH, DH).transpose(0, 2, 1, 3)
    s = np.einsum("bhqd,bhkd->bhqk", q, k) * SCALE
    pad = query_mask[:, None, :, None] & query_mask[:, None, None, :]
    s = np.where(pad, s, -np.inf)
    causal = np.tril(np.ones((Tq, Tq), dtype=bool))
    s = np.where(causal[None, None], s, -np.inf)
    smax = np.max(s, axis=-1, keepdims=True)
    smax = np.where(np.isfinite(smax), smax, 0.0)
    e = np.exp(s - smax)
    lse = np.log(np.sum(e, axis=-1, keepdims=True)) + smax
    slog = s - lse
    t = slog + np.log(attn_prior[:, :Tq][:, None] + TINY)
    tmax = np.max(t, axis=-1, keepdims=True)
    tmax = np.where(np.isfinite(tmax), tmax, 0.0)
    et = np.exp(t - tmax)
    p = et / np.sum(et, axis=-1, keepdims=True)
    p = np.where(pad, p, 0.0)
    y = np.einsum("bhqk,bhkd->bhqd", p, v)
    y = y.transpose(0, 2, 1, 3).reshape(Bq, Tq, NH * DH)
    out = y @ Wo
    return (
        out.astype(np.float32),
        p.astype(np.float32),
        s.astype(np.float32),
    )


def kernel(query, query_mask, attn_prior, Wqkv, Wo):
    query = np.asarray(query)
    query_mask = np.asarray(query_mask)
    attn_prior = np.asarray(attn_prior)
    Wqkv = np.asarray(Wqkv)
    Wo = np.asarray(Wo)
    if not query_mask.all():
        return _numpy_fallback(query, query_mask, attn_prior, Wqkv, Wo)

    r = _get_runner()
    in_maps = make_in_maps(query, attn_prior, Wqkv, Wo)
    args = r.put_inputs(in_maps)
    outs = r.run(args)

    # (8*4, T, T): cores ordered (b, g) with heads ascending -> zero-copy views
    score = np.asarray(outs["score"]).reshape(2, NH, T, T)
    prob = np.asarray(outs["prob"]).reshape(2, NH, T, T)
    outT = np.asarray(outs["outT"]).reshape(8, D, T)
    out = np.stack(
        [
            (outT[0] + outT[1] + outT[2] + outT[3]).T,
            (outT[4] + outT[5] + outT[6] + outT[7]).T,
        ]
    ).astype(np.float32)
    return out, prob, score
